# revision 1
# baseline (speedup 1.0000x reference)
"""Clements-mesh kernel for Trainium2 (8 NeuronCores, data-parallel).

The reference applies 64 layers of 2x2 Givens-like rotations (alternating
even/odd pair offsets) to x [32768, 256].  Each layer is right-multiplication
by a 256x256 block-diagonal orthogonal matrix U_l, so the whole network is
out = x @ (U_0 @ U_1 @ ... @ U_63) = x @ M with M a dense 256x256 matrix that
only depends on the tiny theta [64, 128].  M is built on host in float64;
the device kernel is a single [4096, 256] @ [256, 256] matmul per core,
which is memory-bound (4 MiB in + 4 MiB out per core).

Precision: the PE runs bf16 at 1 cycle/row but fp32 at 4 (and fp32r requires
explicitly rounded TF32-like inputs), so the matmul is done as a 3-term
bf16 split: x = xh + xl, M = Mh + Ml (bf16 each, RTNE), and
out ~= xh@Mh + xh@Ml + xl@Mh accumulated exactly in fp32 PSUM.  The dropped
xl@Ml term is ~2^-18 relative; measured end-to-end rel err vs the reference
is ~4.5e-6 (the reference itself deviates ~2.2e-6 from float64).

Device layout: TensorE contracts over the partition dim of both operands, so
x is shipped feature-major (host pre-transpose), split hi/lo on host:
  xin [4, 128, 256+4096] bf16  (term t = [M-term | x-term] columns; terms:
                                (Mh_kc0|xh_kc0), (Mh_kc1|xh_kc1),
                                (Ml_kc0|xl_kc0), (Ml_kc1|xl_kc1); kc =
                                feature chunk of 128, x free dim = batch)
  outT[2, 128, 4096] f32       (feature chunk jc, feature-in-chunk, batch)
out^T[j, b] = sum_k M[k, j] * x^T[k, b]; PSUM banks are drained to SBUF by
DVE/ACT (DMA cannot read PSUM) and DMAed out feature-major; the host
transposes back while gathering.
"""

import sys

import numpy as np

if "/opt/trn_rl_repo" not in sys.path:
    sys.path.insert(0, "/opt/trn_rl_repo")

import concourse.bass as bass
import concourse.mybir as mybir
from concourse.tile import TileContext

D = 256          # feature dim
B = 32768        # batch
NCORES = 8
BS = B // NCORES  # 4096 batch rows per core
P = 128          # SBUF partitions
NB = 512         # batch columns per matmul (one fp32 PSUM bank)
NBLK = BS // NB  # 8
F32 = mybir.dt.float32
BF16 = mybir.dt.bfloat16

# (x term, M term) pairs accumulated per PSUM bank: hh + hl + lh.
# x terms: 0=xh_kc0, 1=xh_kc1, 2=xl_kc0, 3=xl_kc1; M terms likewise.
TERMS = [(0, 0), (1, 1), (0, 2), (1, 3), (2, 0), (3, 1)]

_NC_CACHE = {}


def _fused_matrix(theta: np.ndarray) -> np.ndarray:
    """M = U_0 @ U_1 @ ... @ U_63 in float64."""
    theta = np.asarray(theta, dtype=np.float64)
    M = np.eye(D, dtype=np.float64)
    for layer in range(theta.shape[0]):
        th = theta[layer]
        if layer % 2 == 0:
            npairs = D // 2
            i_idx = np.arange(0, D - 1, 2)
        else:
            npairs = D // 2 - 1
            i_idx = np.arange(1, D - 2, 2)
        j_idx = i_idx + 1
        c = np.cos(2.0 * th[:npairs])
        s = np.sin(2.0 * th[:npairs])
        Mi = M[:, i_idx].copy()
        Mj = M[:, j_idx]
        M[:, i_idx] = c * Mi + s * Mj
        M[:, j_idx] = s * Mi - c * Mj
    return M


def _split_bf16(a32: np.ndarray):
    """a32 (f32) -> (hi, lo) bf16 with hi + lo ~= a32 (RTNE both)."""
    import ml_dtypes

    hi = a32.astype(ml_dtypes.bfloat16)
    lo = (a32 - hi.astype(np.float32)).astype(ml_dtypes.bfloat16)
    return hi, lo


def _legalize_waits(nc: bass.Bass, max_waits: int = 1) -> None:
    """Split instructions carrying more than ``max_waits`` sync waits.

    This walrus build rejects instructions with multiple sync-wait commands
    (e.g. the Tile tail drain waits on every engine/DMA-lane sem at once).
    Excess waits move to injected same-engine NoOps immediately before the
    instruction, which is semantically identical: the engine blocks on each
    wait in sequence before executing the original instruction.
    """
    for fn in nc.m.functions:
        for blk in fn.blocks:
            insts = blk.instructions
            i = 0
            while i < len(insts):
                inst = insts[i]
                si = inst.sync_info
                if si is not None and len(si.on_wait) > max_waits:
                    waits = list(si.on_wait)
                    keep, extra = waits[-max_waits:], waits[:-max_waits]
                    for k, w in enumerate(extra):
                        nop = mybir.InstNoOp(
                            name=f"{inst.name}-waitsplit-{k}", ins=[], outs=[]
                        )
                        nop.engine = inst.engine
                        nop.sync_info = mybir.SyncInfo(on_wait=[w], on_update=[])
                        insts.insert(i, nop)
                        i += 1
                    inst.sync_info = mybir.SyncInfo(
                        on_wait=keep, on_update=list(si.on_update)
                    )
                i += 1


def _strip_barriers(nc: bass.Bass) -> None:
    """Remove the exit all-engine EVSEM butterfly + drains (~4-7 us).

    The exit barrier only synchronizes engine stream ends; our semaphore
    protocol (SP waits for every out-DMA receipt, GpSimd then resets the
    semaphores) already guarantees completion ordering.  The *init* barrier
    is kept: it orders the GpSimd start-of-run semaphore clears before any
    engine's first wait, making the NEFF robust to dirty device semaphore
    state left by a crashed or foreign predecessor kernel.
    """
    fn = nc.m.functions[0]

    def is_barrier(inst):
        tn = type(inst).__name__
        if tn == "InstDrain":
            return True
        return tn == "InstEventSemaphore" and inst.name.startswith("barrier")

    blk = fn.blocks[-1]
    insts = blk.instructions
    keep = [i for i in insts if not is_barrier(i)]
    if len(keep) != len(insts):
        insts[:] = keep


def _build_nc_raw() -> bass.Bass:
    """Hand-scheduled version: chunked DMA/PE/copy/DMA-out pipeline with
    explicit semaphores, no Tile tail barrier (saves ~25 us vs Tile)."""
    from contextlib import ExitStack

    nc = bass.Bass()
    # xin row t = [M term t (256 cols) | x term t (4096 cols)], bf16.
    xin = nc.declare_dram_parameter("xin", [4, P, D + BS], BF16, isOutput=False)
    outT = nc.declare_dram_parameter("outT", [2, P, BS], F32, isOutput=True)

    # Graded batch chunks: small first chunk so the PE starts early, larger
    # later chunks for DMA efficiency (PE consumes ~2x slower than DMA).
    CHUNKS = [512, 512, 1024, 1024, 1024]
    assert sum(CHUNKS) == BS
    NWARM = 7           # HAM warmup matmuls while the first chunk streams in
    OG = 1              # PSUM banks per out-DMA (256 KB each)

    with ExitStack() as ctx:
        TW = D + BS  # per-term SBUF column stride: [m_t | x_t]
        x_sb = ctx.enter_context(nc.sbuf_tensor("x_sb", [P, 4 * TW], BF16))
        o_sb = ctx.enter_context(nc.sbuf_tensor("o_sb", [P, 2 * BS], F32))
        ps = [
            ctx.enter_context(nc.psum_tensor(f"ps{b}", [P, NB], F32))
            for b in range(8)
        ]
        in_sem = ctx.enter_context(nc.semaphore("in_sem"))
        pe_sem = ctx.enter_context(nc.semaphore("pe_sem"))
        dve_sem = ctx.enter_context(nc.semaphore("dve_sem"))
        act_sem = ctx.enter_context(nc.semaphore("act_sem"))
        out_sem = ctx.enter_context(nc.semaphore("out_sem"))
        start_sem = ctx.enter_context(nc.semaphore("start_sem"))
        block = ctx.enter_context(nc.Block())

        # Group g = 2*bb + jc fills PSUM bank g % 8 with 6 accumulated
        # matmuls; jc0 banks drain on DVE, jc1 banks on ACT.

        @block.sync
        def _(sp):
            # Gate the whole DMA stream on GpSimd's dma_reset + sem clears.
            # (If start_sem itself is stale >= 1 we just lose the gating and
            # run with today's behavior; GpSimd clears it at end-of-run.)
            sp.wait_ge(start_sem, 1)
            # One DMA per (term, batch chunk); chunk 0 also carries the four
            # 256-column M-term blocks packed ahead of the x columns, so the
            # PE can start after just four DMAs.
            off = 0
            for ci, cb in enumerate(CHUNKS):
                lead = D if ci == 0 else 0
                for t in range(4):
                    sp.dma_start(
                        out=x_sb[:, t * TW + D + off - lead : t * TW + D + off + cb],
                        in_=xin[t][:, D + off - lead : D + off + cb],
                    ).then_inc(in_sem, 16)
                off += cb
            # Output DMAs (one per PSUM bank pair and jc, 512 KB each),
            # issued in completion order behind the input stream (FIFO ring).
            for bp in range(NBLK // OG):
                for jc in range(2):
                    sem = dve_sem if jc == 0 else act_sem
                    sp.wait_ge(sem, OG * (bp + 1))
                    lo, hi = bp * OG * NB, (bp + 1) * OG * NB
                    sp.dma_start(
                        out=outT[jc][:, lo:hi],
                        in_=o_sb[:, jc * BS + lo : jc * BS + hi],
                    ).then_inc(out_sem, 16)

        @block.tensor
        def _(pe):
            # Warm the PE HAM clock gate on garbage SBUF while chunk 0 lands;
            # bank 7's real group later overwrites this via start=True.
            for _w in range(NWARM):
                pe.matmul(
                    ps[7][:],
                    lhsT=x_sb[:, 0:P],
                    rhs=x_sb[:, D : D + NB],
                    start=True,
                    stop=True,
                )
            g = 0
            ndma = 0
            off = 0
            for cb in CHUNKS:
                ndma += 4
                pe.wait_ge(in_sem, 16 * ndma)
                for bb in range(off // NB, (off + cb) // NB):
                    for jc in range(2):
                        bank = g % 8
                        if g >= 8:
                            prev = g - 8
                            sem = dve_sem if prev % 2 == 0 else act_sem
                            pe.wait_ge(sem, prev // 2 + 1)
                        mm = None
                        for i, (x_t, m_t) in enumerate(TERMS):
                            mm = pe.matmul(
                                ps[bank][:],
                                lhsT=x_sb[
                                    :, m_t * TW + jc * P : m_t * TW + (jc + 1) * P
                                ],
                                rhs=x_sb[
                                    :,
                                    x_t * TW + D + bb * NB : x_t * TW
                                    + D
                                    + (bb + 1) * NB,
                                ],
                                start=(i == 0),
                                stop=(i == len(TERMS) - 1),
                            )
                        mm.then_inc(pe_sem, 1)
                        g += 1
                off += cb

        @block.vector
        def _(dve):
            # Delay ops: give GpSimd's start-of-run semaphore clears time to
            # land before our first wait could observe stale values.
            dve.memset(o_sb[:, 0:NB], 0.0)
            dve.memset(o_sb[:, 0:NB], 0.0)
            for i in range(NBLK):  # jc0 groups: g = 2i
                dve.wait_ge(pe_sem, 2 * i + 1)
                dve.tensor_copy(
                    o_sb[:, i * NB : (i + 1) * NB], ps[(2 * i) % 8][:]
                ).then_inc(dve_sem, 1)

        @block.scalar
        def _(act):
            # Delay ops, same reason as the DVE memsets.
            act.copy(o_sb[:, BS : BS + NB], o_sb[:, BS : BS + NB])
            act.copy(o_sb[:, BS : BS + NB], o_sb[:, BS : BS + NB])
            for i in range(NBLK):  # jc1 groups: g = 2i + 1
                act.wait_ge(pe_sem, 2 * i + 2)
                act.copy(
                    o_sb[:, BS + i * NB : BS + (i + 1) * NB], ps[(2 * i + 1) % 8][:]
                ).then_inc(act_sem, 1)

        @block.gpsimd
        def _(gp):
            # Start-of-run: drain/reset stale DMA-queue state (an aborted
            # predecessor execution can leave rings mid-flight) and zero our
            # semaphores, then release the SP DMA stream via start_sem.
            gp.dma_reset()
            for s in (in_sem, pe_sem, dve_sem, act_sem, out_sem):
                gp.sem_clear(s)
            gp.sem_inc(start_sem, 1)
            # End-of-run: wait for the last output-DMA write receipt, then
            # reset semaphores so the loaded NEFF is re-executable.
            gp.wait_ge(out_sem, 16 * 2 * (NBLK // OG))
            for s in (in_sem, pe_sem, dve_sem, act_sem, out_sem, start_sem):
                gp.sem_clear(s)

    _strip_barriers(nc)
    _legalize_waits(nc)
    return nc


def _get_nc() -> bass.Bass:
    if "nc" not in _NC_CACHE:
        _NC_CACHE["nc"] = _build_nc_raw()
    return _NC_CACHE["nc"]


def _make_in_maps(x: np.ndarray, theta: np.ndarray):
    x = np.ascontiguousarray(np.asarray(x), dtype=np.float32)
    M32 = _fused_matrix(theta).astype(np.float32)
    mh, ml = _split_bf16(M32)
    m_arr = np.stack(
        [mh[:P], mh[P:], ml[:P], ml[P:]], axis=0
    )  # [4, 128, 256] bf16
    m_arr = np.ascontiguousarray(m_arr)

    xr = x.reshape(NCORES, BS, D)
    in_maps = []
    for c in range(NCORES):
        shard_t = np.ascontiguousarray(xr[c].T)  # [256, 4096] f32
        xh, xl = _split_bf16(shard_t)
        xs = np.stack([xh[:P], xh[P:], xl[:P], xl[P:]], axis=0)
        # Pack the four 256-col M-term blocks ahead of the x columns.
        xin = np.ascontiguousarray(np.concatenate([m_arr, xs], axis=2))
        in_maps.append({"xin": xin})
    return in_maps


def _gather(results) -> np.ndarray:
    out = np.empty((B, D), dtype=np.float32)
    for c in range(NCORES):
        outT = results[c]["outT"].reshape(D, BS)
        out[c * BS : (c + 1) * BS] = outT.T
    return out


def run(x: np.ndarray, theta: np.ndarray, trace: bool = False):
    """Returns (out, BassKernelResults)."""
    from concourse.bass_utils import run_bass_kernel_spmd

    in_maps = _make_in_maps(x, theta)
    res = run_bass_kernel_spmd(
        _get_nc(), in_maps, list(range(NCORES)), trace=trace
    )
    return _gather(res.results), res


def _self_check(x: np.ndarray, out: np.ndarray) -> bool:
    """M is a product of orthogonal factors, so ||out_row|| == ||x_row||.

    A cheap reference-free integrity check that catches the rare transient
    corruption seen when an execution races stale device state (crashed
    predecessor kernel, wedged DMA queues).
    """
    xn = np.linalg.norm(np.asarray(x, dtype=np.float64), axis=1)
    on = np.linalg.norm(out.astype(np.float64), axis=1)
    return bool(np.max(np.abs(on - xn) / np.maximum(xn, 1e-6)) < 1e-3)


def kernel(x: np.ndarray, theta: np.ndarray) -> np.ndarray:
    for attempt in range(3):
        out, _ = run(x, theta, trace=False)
        if _self_check(x, out):
            return out
    return out



# revision 2
# speedup vs baseline: 1.4231x; 1.4231x over previous
"""Clements-mesh kernel for Trainium2 (8 NeuronCores, data-parallel).

The reference applies 64 layers of 2x2 Givens-like rotations (alternating
even/odd pair offsets) to x [32768, 256].  Each layer is right-multiplication
by a 256x256 block-diagonal orthogonal matrix U_l, so the whole network is
out = x @ (U_0 @ U_1 @ ... @ U_63) = x @ M with M a dense 256x256 matrix that
only depends on the tiny theta [64, 128].  M is built on host in float64;
the device kernel is a single [4096, 256] @ [256, 256] matmul per core.

Precision: the correctness gate is rel_err < 2e-2, so both x and M are sent
as single bf16 (RTNE) and the result is rounded to bf16 before the output
DMA; accumulation is exact f32 in PSUM.  Measured end-to-end rel err vs the
reference is ~2.9e-3 (7x margin).  This halves HBM traffic vs an x-hi/lo
split with f32 output: 2.2 MiB in + 2.1 MiB out per core, ~12 us at the
~360 GB/s per-core DMA roofline, which is what the kernel is bound by.

Device layout: TensorE contracts over the partition dim of both operands, so
x is shipped feature-major (host pre-transpose) and column-packed in DMA
stream order so every input chunk is ONE contiguous DMA:
  xin [128, 8704] bf16 = [M_kc0 | M_kc1 | X0_kc0 | X0_kc1 | ... | X4_kc1]
where kc = contraction chunk of 128 features and Xi are batch-column chunks
of width CHUNK_W[i].  out^T[j, b] = sum_k M[k, j] x^T[k, b] accumulates over
kc0+kc1 into one PSUM bank per (512-batch block, output-feature half); banks
are drained (with f32->bf16 cast) to SBUF by DVE (jc0) / ACT (jc1) since DMA
cannot read PSUM, then DMAed out feature-major; the host transposes back and
upcasts to f32 while gathering.

Scheduling: hand-built engine programs with explicit semaphores, no Tile
barriers.  The all-engine init barrier + dma_reset of earlier versions
(~3.5 us) is replaced by a semaphore gate: GpSimd clears the data semaphores
then raises start_sem; everything except the first input DMA (receipted on
its own never-start-cleared c0_sem) is gated behind it.  End-of-run GpSimd
clears make the NEFF re-executable; a reference-free row-norm self-check
with retry in kernel() guards the rare stale-device-state corruption.
"""

import sys

import numpy as np

if "/opt/trn_rl_repo" not in sys.path:
    sys.path.insert(0, "/opt/trn_rl_repo")

import concourse.bass as bass
import concourse.mybir as mybir
from concourse.tile import TileContext

D = 256          # feature dim
B = 32768        # batch
NCORES = 8
BS = B // NCORES  # 4096 batch rows per core
P = 128          # SBUF partitions
NB = 512         # batch columns per matmul (one fp32 PSUM bank)
NBLK = BS // NB  # 8 batch blocks
F32 = mybir.dt.float32
BF16 = mybir.dt.bfloat16

# Batch-column chunks; chunk 0 rides in the same DMA as the two 256-col
# M blocks (small so the PE starts early, later chunks larger for DMA
# efficiency).  Each chunk i contributes cols [off, off+w) for BOTH
# contraction halves, packed kc0-then-kc1, so it is one contiguous DMA.
CHUNK_W = [512, 1024, 1024, 1024, 512]
assert sum(CHUNK_W) == BS
XIN_W = 2 * D + 2 * BS  # 8704

# xin column offset where chunk i's kc0 block starts.
_CS = []
_off = 2 * D
for _w in CHUNK_W:
    _CS.append(_off)
    _off += 2 * _w

# batch block bb (512 cols) -> (chunk index, col offset inside the chunk)
_BB_CHUNK = []
_off = 0
for _ci, _w in enumerate(CHUNK_W):
    for _j in range(_w // NB):
        _BB_CHUNK.append((_ci, _j * NB))
    _off += _w


def _xcol(bb: int, kc: int) -> int:
    ci, off = _BB_CHUNK[bb]
    return _CS[ci] + kc * CHUNK_W[ci] + off


_NC_CACHE = {}


def _fused_matrix(theta: np.ndarray) -> np.ndarray:
    """M = U_0 @ U_1 @ ... @ U_63 in float64."""
    theta = np.asarray(theta, dtype=np.float64)
    M = np.eye(D, dtype=np.float64)
    for layer in range(theta.shape[0]):
        th = theta[layer]
        if layer % 2 == 0:
            npairs = D // 2
            i_idx = np.arange(0, D - 1, 2)
        else:
            npairs = D // 2 - 1
            i_idx = np.arange(1, D - 2, 2)
        j_idx = i_idx + 1
        c = np.cos(2.0 * th[:npairs])
        s = np.sin(2.0 * th[:npairs])
        Mi = M[:, i_idx].copy()
        Mj = M[:, j_idx]
        M[:, i_idx] = c * Mi + s * Mj
        M[:, j_idx] = s * Mi - c * Mj
    return M


def _legalize_waits(nc: bass.Bass, max_waits: int = 1) -> None:
    """Split instructions carrying more than ``max_waits`` sync waits.

    This walrus build rejects instructions with multiple sync-wait commands.
    Excess waits move to injected same-engine NoOps immediately before the
    instruction, which is semantically identical: the engine blocks on each
    wait in sequence before executing the original instruction.
    """
    for fn in nc.m.functions:
        for blk in fn.blocks:
            insts = blk.instructions
            i = 0
            while i < len(insts):
                inst = insts[i]
                si = inst.sync_info
                if si is not None and len(si.on_wait) > max_waits:
                    waits = list(si.on_wait)
                    keep, extra = waits[-max_waits:], waits[:-max_waits]
                    for k, w in enumerate(extra):
                        nop = mybir.InstNoOp(
                            name=f"{inst.name}-waitsplit-{k}", ins=[], outs=[]
                        )
                        nop.engine = inst.engine
                        nop.sync_info = mybir.SyncInfo(on_wait=[w], on_update=[])
                        insts.insert(i, nop)
                        i += 1
                    inst.sync_info = mybir.SyncInfo(
                        on_wait=keep, on_update=list(si.on_update)
                    )
                i += 1


def _strip_barriers(nc: bass.Bass) -> None:
    """Remove ALL all-engine EVSEM barrier butterflies + drains.

    Ordering is carried entirely by our semaphore protocol: GpSimd's
    start-of-run semaphore clears gate every semaphore producer via
    start_sem (the one ungated input DMA receipts on c0_sem, which is
    never start-cleared), and GpSimd's end-of-run clears run after the
    final output-DMA write receipt.
    """
    for fn in nc.m.functions:
        for blk in fn.blocks:
            insts = blk.instructions
            keep = [
                i
                for i in insts
                if not (
                    type(i).__name__ == "InstDrain"
                    or (
                        type(i).__name__ == "InstEventSemaphore"
                        and i.name.startswith("barrier")
                    )
                )
            ]
            if len(keep) != len(insts):
                insts[:] = keep


def _build_nc_raw() -> bass.Bass:
    from contextlib import ExitStack

    nc = bass.Bass()
    xin = nc.declare_dram_parameter("xin", [P, XIN_W], BF16, isOutput=False)
    outT = nc.declare_dram_parameter("outT", [2, P, BS], BF16, isOutput=True)

    NWARM = 7  # HAM/p-state warmup matmuls while the first chunk streams in
    OG = 2     # PSUM banks (per jc) per out-DMA -> 256 KB transfers

    with ExitStack() as ctx:
        x_sb = ctx.enter_context(nc.sbuf_tensor("x_sb", [P, XIN_W], BF16))
        o_sb = ctx.enter_context(nc.sbuf_tensor("o_sb", [P, 2 * BS], BF16))
        ps = [
            ctx.enter_context(nc.psum_tensor(f"ps{b}", [P, NB], F32))
            for b in range(8)
        ]
        c0_sem = ctx.enter_context(nc.semaphore("c0_sem"))
        in_sem = ctx.enter_context(nc.semaphore("in_sem"))
        pe_sem = ctx.enter_context(nc.semaphore("pe_sem"))
        dve_sem = ctx.enter_context(nc.semaphore("dve_sem"))
        act_sem = ctx.enter_context(nc.semaphore("act_sem"))
        out_sem = ctx.enter_context(nc.semaphore("out_sem"))
        start_sem = ctx.enter_context(nc.semaphore("start_sem"))
        block = ctx.enter_context(nc.Block())

        # Group g = 2*bb + jc fills PSUM bank g % 8 with kc0+kc1 accumulated
        # matmuls; jc0 banks drain on DVE, jc1 banks on ACT (f32 -> bf16).

        @block.sync
        def _(sp):
            # Chunk 0 (M blocks + first 512 batch cols) goes out immediately,
            # receipted on c0_sem which GpSimd never clears at start-of-run,
            # so the start_sem gate cannot erase its receipts.
            sp.dma_start(
                out=x_sb[:, 0 : _CS[1]], in_=xin[:, 0 : _CS[1]]
            ).then_inc(c0_sem, 16)
            # Everything else waits for GpSimd's semaphore clears.
            sp.wait_ge(start_sem, 1)
            for ci in range(1, len(CHUNK_W)):
                lo = _CS[ci]
                hi = _CS[ci] + 2 * CHUNK_W[ci] if ci + 1 < len(CHUNK_W) else XIN_W
                sp.dma_start(out=x_sb[:, lo:hi], in_=xin[:, lo:hi]).then_inc(
                    in_sem, 16
                )
            # Output DMAs (one per OG drained banks and jc, 256 KB each),
            # issued in drain-completion order behind the input stream.
            for bp in range(NBLK // OG):
                for jc in range(2):
                    sem = dve_sem if jc == 0 else act_sem
                    sp.wait_ge(sem, OG * (bp + 1))
                    lo, hi = bp * OG * NB, (bp + 1) * OG * NB
                    sp.dma_start(
                        out=outT[jc][:, lo:hi],
                        in_=o_sb[:, jc * BS + lo : jc * BS + hi],
                    ).then_inc(out_sem, 16)

        @block.tensor
        def _(pe):
            # Warm the PE p-state on garbage SBUF while chunk 0 lands; bank
            # 7's real group later overwrites this via start=True.
            for _w in range(NWARM):
                pe.matmul(
                    ps[7][:],
                    lhsT=x_sb[:, 0:P],
                    rhs=x_sb[:, _CS[0] : _CS[0] + NB],
                    start=True,
                    stop=True,
                )
            # Never produce a pe_sem increment before GpSimd's clears are
            # done (the c0 DMA alone could otherwise race them).
            pe.wait_ge(start_sem, 1)
            last_wait = -1
            for bb in range(NBLK):
                ci = _BB_CHUNK[bb][0]
                if ci > last_wait:
                    if ci == 0:
                        pe.wait_ge(c0_sem, 16)
                    else:
                        pe.wait_ge(in_sem, 16 * ci)
                    last_wait = ci
                for jc in range(2):
                    g = 2 * bb + jc
                    if g >= 8:
                        prev = g - 8
                        sem = dve_sem if prev % 2 == 0 else act_sem
                        pe.wait_ge(sem, prev // 2 + 1)
                    pe.matmul(
                        ps[g % 8][:],
                        lhsT=x_sb[:, jc * P : (jc + 1) * P],
                        rhs=x_sb[:, _xcol(bb, 0) : _xcol(bb, 0) + NB],
                        start=True,
                        stop=False,
                    )
                    pe.matmul(
                        ps[g % 8][:],
                        lhsT=x_sb[:, D + jc * P : D + (jc + 1) * P],
                        rhs=x_sb[:, _xcol(bb, 1) : _xcol(bb, 1) + NB],
                        start=False,
                        stop=True,
                    ).then_inc(pe_sem, 1)

        @block.vector
        def _(dve):
            # Tiny delay op: give GpSimd's start-of-run clears time to land
            # before our first wait could observe stale values.
            dve.memset(o_sb[:, 0:8], 0.0)
            for i in range(NBLK):  # jc0 groups: g = 2i
                dve.wait_ge(pe_sem, 2 * i + 1)
                dve.tensor_copy(
                    o_sb[:, i * NB : (i + 1) * NB], ps[(2 * i) % 8][:]
                ).then_inc(dve_sem, 1)

        @block.scalar
        def _(act):
            # Tiny delay op; also triggers the one-time ACT table load well
            # before the first real drain needs it.
            act.copy(o_sb[:, BS : BS + 8], o_sb[:, BS : BS + 8])
            for i in range(NBLK):  # jc1 groups: g = 2i + 1
                act.wait_ge(pe_sem, 2 * i + 2)
                act.copy(
                    o_sb[:, BS + i * NB : BS + (i + 1) * NB], ps[(2 * i + 1) % 8][:]
                ).then_inc(act_sem, 1)

        @block.gpsimd
        def _(gp):
            # Start-of-run: zero the data semaphores, then release everything
            # via start_sem.  (c0_sem deliberately not cleared here.)
            for s in (in_sem, pe_sem, dve_sem, act_sem, out_sem):
                gp.sem_clear(s)
            gp.sem_inc(start_sem, 1)
            # End-of-run: wait for the last output-DMA write receipt, then
            # reset all semaphores so the loaded NEFF is re-executable.
            gp.wait_ge(out_sem, 16 * 2 * (NBLK // OG))
            for s in (c0_sem, in_sem, pe_sem, dve_sem, act_sem, out_sem, start_sem):
                gp.sem_clear(s)

    _strip_barriers(nc)
    _legalize_waits(nc)
    return nc


def _get_nc() -> bass.Bass:
    if "nc" not in _NC_CACHE:
        _NC_CACHE["nc"] = _build_nc_raw()
    return _NC_CACHE["nc"]


def _make_in_maps(x: np.ndarray, theta: np.ndarray):
    import ml_dtypes

    x = np.ascontiguousarray(np.asarray(x), dtype=np.float32)
    mh = _fused_matrix(theta).astype(np.float32).astype(ml_dtypes.bfloat16)

    xr = x.reshape(NCORES, BS, D)
    in_maps = []
    for c in range(NCORES):
        xt = np.ascontiguousarray(xr[c].T).astype(ml_dtypes.bfloat16)
        cols = [mh[:P], mh[P:]]
        off = 0
        for w in CHUNK_W:
            cols.append(xt[:P, off : off + w])
            cols.append(xt[P:, off : off + w])
            off += w
        in_maps.append({"xin": np.ascontiguousarray(np.concatenate(cols, axis=1))})
    return in_maps


def _gather(results) -> np.ndarray:
    out = np.empty((B, D), dtype=np.float32)
    for c in range(NCORES):
        oT = np.asarray(results[c]["outT"])  # [2, 128, 4096] bf16
        out[c * BS : (c + 1) * BS, :P] = oT[0].T.astype(np.float32)
        out[c * BS : (c + 1) * BS, P:] = oT[1].T.astype(np.float32)
    return out


def run(x: np.ndarray, theta: np.ndarray, trace: bool = False):
    """Returns (out, BassKernelResults)."""
    from concourse.bass_utils import run_bass_kernel_spmd

    in_maps = _make_in_maps(x, theta)
    res = run_bass_kernel_spmd(
        _get_nc(), in_maps, list(range(NCORES)), trace=trace
    )
    return _gather(res.results), res


def _self_check(x: np.ndarray, out: np.ndarray) -> bool:
    """M is a product of orthogonal factors, so ||out_row|| == ||x_row||.

    A cheap reference-free integrity check that catches the rare transient
    corruption seen when an execution races stale device state.  The bf16
    pipeline keeps the max row-norm deviation ~1.1e-3; real corruption is
    orders of magnitude larger.
    """
    xn = np.linalg.norm(np.asarray(x, dtype=np.float64), axis=1)
    on = np.linalg.norm(out.astype(np.float64), axis=1)
    return bool(np.max(np.abs(on - xn) / np.maximum(xn, 1e-6)) < 5e-3)


def kernel(x: np.ndarray, theta: np.ndarray) -> np.ndarray:
    for attempt in range(3):
        out, _ = run(x, theta, trace=False)
        if _self_check(x, out):
            return out
    return out


# revision 7
# speedup vs baseline: 1.5839x; 1.1130x over previous
"""Clements-mesh kernel for Trainium2 (8 NeuronCores, data-parallel).

The reference applies 64 layers of 2x2 Givens-like rotations (alternating
even/odd pair offsets) to x [32768, 256].  Each layer is right-multiplication
by a 256x256 block-diagonal orthogonal matrix U_l, so the whole network is
out = x @ (U_0 @ U_1 @ ... @ U_63) = x @ M with M a dense 256x256 matrix that
only depends on the tiny theta [64, 128].  M is built on host in float64;
the device kernel is a single [4096, 256] @ [256, 256] matmul per core.

Precision: the correctness gate is rel_err < 2e-2, so both x and M are sent
as single bf16 (RTNE) and the result is rounded to bf16 before the output
DMA; accumulation is exact f32 in PSUM.  Measured end-to-end rel err vs the
reference is ~2.9e-3 (7x margin).  This halves HBM traffic vs an x-hi/lo
split with f32 output: 2.2 MiB in + 2.1 MiB out per core, ~12 us at the
~360 GB/s per-core DMA roofline, which is what the kernel is bound by.

Device layout: TensorE contracts over the partition dim of both operands, so
x is shipped feature-major (host pre-transpose) and column-packed in DMA
stream order so every input chunk is ONE contiguous DMA:
  xin [128, 8704] bf16 = [M_kc0 | M_kc1 | X0_kc0 | X0_kc1 | ... | X4_kc1]
where kc = contraction chunk of 128 features and Xi are batch-column chunks
of width CHUNK_W[i].  out^T[j, b] = sum_k M[k, j] x^T[k, b] accumulates over
kc0+kc1 into one PSUM bank per (512-batch block, output-feature half); banks
are drained (with f32->bf16 cast) to SBUF by DVE (jc0) / ACT (jc1) since DMA
cannot read PSUM, then DMAed out feature-major; the host transposes back and
upcasts to f32 while gathering.

Scheduling: hand-built engine programs with explicit semaphores, no Tile
barriers.  The all-engine init barrier + dma_reset of earlier versions
(~3.5 us) is replaced by a semaphore gate: GpSimd clears the data semaphores
then raises start_sem; everything except the first input DMA (receipted on
its own never-start-cleared c0_sem) is gated behind it.  End-of-run GpSimd
clears make the NEFF re-executable; a reference-free row-norm self-check
with retry in kernel() guards the rare stale-device-state corruption.
"""

import sys

import numpy as np

if "/opt/trn_rl_repo" not in sys.path:
    sys.path.insert(0, "/opt/trn_rl_repo")

import concourse.bass as bass
import concourse.mybir as mybir
from concourse.tile import TileContext

D = 256          # feature dim
B = 32768        # batch
NCORES = 8
BS = B // NCORES  # 4096 batch rows per core
P = 128          # SBUF partitions
NB = 512         # batch columns per matmul (one fp32 PSUM bank)
NBLK = BS // NB  # 8 batch blocks
F32 = mybir.dt.float32
BF16 = mybir.dt.bfloat16

# Batch-column chunks; chunk 0 rides in the same DMA as the two 256-col
# M blocks (small so the PE starts early, later chunks larger for DMA
# efficiency).  Each chunk i contributes cols [off, off+w) for BOTH
# contraction halves, packed kc0-then-kc1, so it is one contiguous DMA.
CHUNK_W = [512, 1024, 1024, 1024, 512]
assert sum(CHUNK_W) == BS
XIN_W = 2 * D + 2 * BS  # 8704

# xin column offset where chunk i's kc0 block starts.
_CS = []
_off = 2 * D
for _w in CHUNK_W:
    _CS.append(_off)
    _off += 2 * _w

# batch block bb (512 cols) -> (chunk index, col offset inside the chunk)
_BB_CHUNK = []
_off = 0
for _ci, _w in enumerate(CHUNK_W):
    for _j in range(_w // NB):
        _BB_CHUNK.append((_ci, _j * NB))
    _off += _w


def _xcol(bb: int, kc: int) -> int:
    ci, off = _BB_CHUNK[bb]
    return _CS[ci] + kc * CHUNK_W[ci] + off


_NC_CACHE = {}


def _fused_matrix(theta: np.ndarray) -> np.ndarray:
    """M = U_0 @ U_1 @ ... @ U_63 in float64."""
    theta = np.asarray(theta, dtype=np.float64)
    M = np.eye(D, dtype=np.float64)
    for layer in range(theta.shape[0]):
        th = theta[layer]
        if layer % 2 == 0:
            npairs = D // 2
            i_idx = np.arange(0, D - 1, 2)
        else:
            npairs = D // 2 - 1
            i_idx = np.arange(1, D - 2, 2)
        j_idx = i_idx + 1
        c = np.cos(2.0 * th[:npairs])
        s = np.sin(2.0 * th[:npairs])
        Mi = M[:, i_idx].copy()
        Mj = M[:, j_idx]
        M[:, i_idx] = c * Mi + s * Mj
        M[:, j_idx] = s * Mi - c * Mj
    return M


def _legalize_waits(nc: bass.Bass, max_waits: int = 1) -> None:
    """Split instructions carrying more than ``max_waits`` sync waits.

    This walrus build rejects instructions with multiple sync-wait commands.
    Excess waits move to injected same-engine NoOps immediately before the
    instruction, which is semantically identical: the engine blocks on each
    wait in sequence before executing the original instruction.
    """
    for fn in nc.m.functions:
        for blk in fn.blocks:
            insts = blk.instructions
            i = 0
            while i < len(insts):
                inst = insts[i]
                si = inst.sync_info
                if si is not None and len(si.on_wait) > max_waits:
                    waits = list(si.on_wait)
                    keep, extra = waits[-max_waits:], waits[:-max_waits]
                    for k, w in enumerate(extra):
                        nop = mybir.InstNoOp(
                            name=f"{inst.name}-waitsplit-{k}", ins=[], outs=[]
                        )
                        nop.engine = inst.engine
                        nop.sync_info = mybir.SyncInfo(on_wait=[w], on_update=[])
                        insts.insert(i, nop)
                        i += 1
                    inst.sync_info = mybir.SyncInfo(
                        on_wait=keep, on_update=list(si.on_update)
                    )
                i += 1


def _strip_barriers(nc: bass.Bass) -> None:
    """Remove ALL all-engine EVSEM barrier butterflies + drains.

    Ordering is carried entirely by our semaphore protocol: GpSimd's
    start-of-run semaphore clears gate every semaphore producer via
    start_sem (the one ungated input DMA receipts on c0_sem, which is
    never start-cleared), and GpSimd's end-of-run clears run after the
    final output-DMA write receipt.
    """
    for fn in nc.m.functions:
        for blk in fn.blocks:
            insts = blk.instructions
            keep = [
                i
                for i in insts
                if not (
                    type(i).__name__ == "InstDrain"
                    or (
                        type(i).__name__ == "InstEventSemaphore"
                        and i.name.startswith("barrier")
                    )
                )
            ]
            if len(keep) != len(insts):
                insts[:] = keep


def _build_nc_raw() -> bass.Bass:
    from contextlib import ExitStack

    nc = bass.Bass()
    xin = nc.declare_dram_parameter("xin", [P, XIN_W], BF16, isOutput=False)
    outT = nc.declare_dram_parameter("outT", [2, P, BS], BF16, isOutput=True)

    NWARM = 5  # HAM/p-state warmup matmuls while the first chunk streams in
    # PSUM banks (per jc) per out-DMA; tapered so the final transfer (which
    # the kernel-end drain effectively waits behind) is a single 128 KB bank.
    OGS = [2, 2, 2, 1, 1]
    assert sum(OGS) == NBLK

    with ExitStack() as ctx:
        x_sb = ctx.enter_context(nc.sbuf_tensor("x_sb", [P, XIN_W], BF16))
        o_sb = ctx.enter_context(nc.sbuf_tensor("o_sb", [P, 2 * BS], BF16))
        ps = [
            ctx.enter_context(nc.psum_tensor(f"ps{b}", [P, NB], F32))
            for b in range(8)
        ]
        c0_sem = ctx.enter_context(nc.semaphore("c0_sem"))
        in_sem = ctx.enter_context(nc.semaphore("in_sem"))
        pe_sem = ctx.enter_context(nc.semaphore("pe_sem"))
        dve_sem = ctx.enter_context(nc.semaphore("dve_sem"))
        act_sem = ctx.enter_context(nc.semaphore("act_sem"))
        out_sem = ctx.enter_context(nc.semaphore("out_sem"))
        start_sem = ctx.enter_context(nc.semaphore("start_sem"))
        block = ctx.enter_context(nc.Block())

        # Group g = 2*bb + jc fills PSUM bank g % 8 with kc0+kc1 accumulated
        # matmuls; jc0 banks drain on DVE, jc1 banks on ACT (f32 -> bf16).

        @block.sync
        def _(sp):
            # Chunk 0 (M blocks + first 512 batch cols) goes out immediately,
            # receipted on c0_sem which GpSimd never clears at start-of-run,
            # so the start_sem gate cannot erase its receipts.
            sp.dma_start(
                out=x_sb[:, 0 : _CS[1]], in_=xin[:, 0 : _CS[1]]
            ).then_inc(c0_sem, 16)
            # Everything else waits for GpSimd's semaphore clears.
            sp.wait_ge(start_sem, 1)
            for ci in range(1, len(CHUNK_W)):
                lo = _CS[ci]
                hi = _CS[ci] + 2 * CHUNK_W[ci] if ci + 1 < len(CHUNK_W) else XIN_W
                sp.dma_start(out=x_sb[:, lo:hi], in_=xin[:, lo:hi]).then_inc(
                    in_sem, 16
                )
            # Output DMAs issued in drain-completion order behind the input
            # stream.  Receipts land on out_sem which nothing waits on
            # (walrus requires a completion semaphore): the SP queue itself
            # retires only after the last pseudo-DMA transfer, and the
            # runtime's end-of-execution teardown quiesces the DMA path
            # before results are read.
            done = 0
            for og in OGS:
                for jc in range(2):
                    sem = dve_sem if jc == 0 else act_sem
                    sp.wait_ge(sem, done + og)
                    lo, hi = done * NB, (done + og) * NB
                    sp.dma_start(
                        out=outT[jc][:, lo:hi],
                        in_=o_sb[:, jc * BS + lo : jc * BS + hi],
                    ).then_inc(out_sem, 16)
                done += og

        @block.tensor
        def _(pe):
            # Warm the PE p-state on garbage SBUF while chunk 0 lands; bank
            # 7's real group later overwrites this via start=True.
            for _w in range(NWARM):
                pe.matmul(
                    ps[7][:],
                    lhsT=x_sb[:, 0:P],
                    rhs=x_sb[:, _CS[0] : _CS[0] + NB],
                    start=True,
                    stop=True,
                )
            # Never produce a pe_sem increment before GpSimd's clears are
            # done (the c0 DMA alone could otherwise race them).
            pe.wait_ge(start_sem, 1)
            last_wait = -1
            for bb in range(NBLK):
                ci = _BB_CHUNK[bb][0]
                if ci > last_wait:
                    if ci == 0:
                        pe.wait_ge(c0_sem, 16)
                    else:
                        pe.wait_ge(in_sem, 16 * ci)
                    last_wait = ci
                for jc in range(2):
                    g = 2 * bb + jc
                    if g >= 8:
                        prev = g - 8
                        sem = dve_sem if prev % 2 == 0 else act_sem
                        pe.wait_ge(sem, prev // 2 + 1)
                    pe.matmul(
                        ps[g % 8][:],
                        lhsT=x_sb[:, jc * P : (jc + 1) * P],
                        rhs=x_sb[:, _xcol(bb, 0) : _xcol(bb, 0) + NB],
                        start=True,
                        stop=False,
                    )
                    pe.matmul(
                        ps[g % 8][:],
                        lhsT=x_sb[:, D + jc * P : D + (jc + 1) * P],
                        rhs=x_sb[:, _xcol(bb, 1) : _xcol(bb, 1) + NB],
                        start=False,
                        stop=True,
                    ).then_inc(pe_sem, 1)

        @block.vector
        def _(dve):
            # Tiny delay op: give GpSimd's start-of-run clears time to land
            # before our first wait could observe stale values.
            dve.memset(o_sb[:, 0:8], 0.0)
            for i in range(NBLK):  # jc0 groups: g = 2i
                dve.wait_ge(pe_sem, 2 * i + 1)
                dve.tensor_copy(
                    o_sb[:, i * NB : (i + 1) * NB], ps[(2 * i) % 8][:]
                ).then_inc(dve_sem, 1)

        @block.scalar
        def _(act):
            # Tiny delay op; also triggers the one-time ACT table load well
            # before the first real drain needs it.
            act.copy(o_sb[:, BS : BS + 8], o_sb[:, BS : BS + 8])
            for i in range(NBLK):  # jc1 groups: g = 2i + 1
                act.wait_ge(pe_sem, 2 * i + 2)
                act.copy(
                    o_sb[:, BS + i * NB : BS + (i + 1) * NB], ps[(2 * i + 1) % 8][:]
                ).then_inc(act_sem, 1)

        @block.gpsimd
        def _(gp):
            # Start-of-run: zero the data semaphores, then release everything
            # via start_sem.  (c0_sem deliberately not cleared here: its DMA
            # is dispatched ungated, so a start-clear could erase in-flight
            # receipts.)
            for s in (in_sem, pe_sem, dve_sem, act_sem, out_sem):
                gp.sem_clear(s)
            gp.sem_inc(start_sem, 1)
            # End-of-run: once the last drains are done (i.e. every sem this
            # NEFF waits on has passed its final wait), reset the two
            # semaphores that are NOT start-of-run-cleared so the NEFF is
            # re-executable.  Finishes under the shadow of the final output
            # DMAs still retiring on the SP queue.
            gp.wait_ge(dve_sem, NBLK)
            gp.wait_ge(act_sem, NBLK)
            gp.sem_clear(c0_sem)
            gp.sem_clear(start_sem)

    _strip_barriers(nc)
    _legalize_waits(nc)
    return nc


def _get_nc() -> bass.Bass:
    if "nc" not in _NC_CACHE:
        _NC_CACHE["nc"] = _build_nc_raw()
    return _NC_CACHE["nc"]


def _make_in_maps(x: np.ndarray, theta: np.ndarray):
    import ml_dtypes

    x = np.ascontiguousarray(np.asarray(x), dtype=np.float32)
    mh = _fused_matrix(theta).astype(np.float32).astype(ml_dtypes.bfloat16)

    xr = x.reshape(NCORES, BS, D)
    in_maps = []
    for c in range(NCORES):
        xt = np.ascontiguousarray(xr[c].T).astype(ml_dtypes.bfloat16)
        cols = [mh[:P], mh[P:]]
        off = 0
        for w in CHUNK_W:
            cols.append(xt[:P, off : off + w])
            cols.append(xt[P:, off : off + w])
            off += w
        in_maps.append({"xin": np.ascontiguousarray(np.concatenate(cols, axis=1))})
    return in_maps


def _gather(results) -> np.ndarray:
    out = np.empty((B, D), dtype=np.float32)
    for c in range(NCORES):
        oT = np.asarray(results[c]["outT"])  # [2, 128, 4096] bf16
        out[c * BS : (c + 1) * BS, :P] = oT[0].T.astype(np.float32)
        out[c * BS : (c + 1) * BS, P:] = oT[1].T.astype(np.float32)
    return out


def run(x: np.ndarray, theta: np.ndarray, trace: bool = False):
    """Returns (out, BassKernelResults)."""
    from concourse.bass_utils import run_bass_kernel_spmd

    in_maps = _make_in_maps(x, theta)
    res = run_bass_kernel_spmd(
        _get_nc(), in_maps, list(range(NCORES)), trace=trace
    )
    return _gather(res.results), res


def _self_check(x: np.ndarray, out: np.ndarray) -> bool:
    """M is a product of orthogonal factors, so ||out_row|| == ||x_row||.

    A cheap reference-free integrity check that catches the rare transient
    corruption seen when an execution races stale device state.  The bf16
    pipeline keeps the max row-norm deviation ~1.1e-3; real corruption is
    orders of magnitude larger.
    """
    xn = np.linalg.norm(np.asarray(x, dtype=np.float64), axis=1)
    on = np.linalg.norm(out.astype(np.float64), axis=1)
    return bool(np.max(np.abs(on - xn) / np.maximum(xn, 1e-6)) < 5e-3)


def kernel(x: np.ndarray, theta: np.ndarray) -> np.ndarray:
    for attempt in range(3):
        out, _ = run(x, theta, trace=False)
        if _self_check(x, out):
            return out
    return out
